# revision 5
# baseline (speedup 1.0000x reference)
"""Trainium2 Bass kernel for the pixel-RNN (tanh RNN, T=784, H=512, B=256).

Strategy: data-parallel over batch (32 samples per core, 8 cores); the
sequential 784-step loop runs locally per core; tiny classifier head on
device, log-softmax/loss/argmax on host from exact fp32 logits.

Per-step structure (all matmul operands fp16, PSUM accumulates f32):
  - z PSUM [128, 128]: 4-way column-strip packing; strip j (partitions
    32j..32j+32, tile_position (0,32j)) holds z[b, 128j+n].
  - x-term: per strip, one K=2 packed matmul [x_t; 1]^T @ [w_ih; b]
    (start=True), emitted one step early to fill the ACT/ST bubble.
  - recurrence: 4 slots (kc) x 4 packed strips, lhsT = a [128, 32]
    column slice of the block-transposed state (see below), rhs = a
    row-permuted W_hh^T block; each slot streams 128 cols (~100ns).
  - tanh: ONE ScalarE ACTIVATE [128,128] (~368ns). The ACT engine has
    exec-queue depth 0, so splitting the tanh serializes with an extra
    ~330ns const per piece - measured net loss.
  - layout fix: ONE DVE StreamTranspose pair (cols [0:64], [64:128]).
    ST semantics: STout[32r+a, 32c+d] = hh[32r+d, 32c+a], so the slice
    STout[:, 32kc:32kc+32] equals h[b, 128r + 32kc + a] - directly a
    valid matmul stationary for the H-subset {128r+32kc+a}. W_hh /
    lin_W rows are permuted host-side to match (exact relabeling).
    This replaces v1's PE transpose + 2 PSUM->SBUF vector copies
    (~800ns of chain latency) with ~390ns of DVE work; the 2-way ST
    split lets slots kc0/kc1 start ~70ns before ST2 retires (DVE
    queue depth 8 pipelines, unlike ACT).

Measured steady state (fast clock): ACT 368 + write-ack 215 + ST1 244 +
hop 54 + LDW 101 + 4 slot streams ~500 + tail 88 = ~1582ns/step. The
step is a latency chain, not a throughput problem: every engine idles
>60%; batch-splitting/core-pipelining cannot help because every piece's
cost is batch-size-independent and the 784 steps are strictly serial.

Clock-state pitfall: the whole core (PE/ACT/DVE alike) runs 1.2x slower
until sustained dense load up-shifts it, and it decays back after ~10
min idle. The RNN chain alone (<40% duty) never triggers the up-shift,
so a cold run stays 1.2x slow end-to-end (v1's 1457us "good" runs vs
1747us cold runs - same NEFF). A ~36us dense matmul prologue inside the
NEFF forces the fast state deterministically.

Sem plumbing: updates of same-flavor matmul runs are folded onto the
run's last member ACROSS interleaved wait-free LDWEIGHTS (the LDW+MM
emission interleave otherwise defeats folding and each MM's ~26ns EVT
write delays the ACT's wait resolution); multi-wait instructions are
split so each HW instruction carries exactly one wait (codegen limit).

History: v1 (PE transpose + copies) 1457694ns warm / 1747679 cold ->
v2 (StreamTranspose layout + fold-across-LDW + warmup) 1292013ns
(1259299 without the 33us warmup premium).
"""

import sys

if "/opt/trn_rl_repo" not in sys.path:
    sys.path.insert(0, "/opt/trn_rl_repo")

import numpy as np

B, T, H, NCLS = 256, 784, 512, 10
NCORES = 8
BC = B // NCORES   # 32 samples per core
KC = H // 128      # 4 contraction slots
N_WARM = 300       # dense prologue slot-groups (~36us) to up-shift the clock

_BUILD_CACHE = {}


def _build(t_steps=T):
    import concourse.bass as bass
    import concourse.mybir as mybir
    from concourse import tile

    f32 = mybir.dt.float32
    f16 = mybir.dt.float16
    Tanh = mybir.ActivationFunctionType.Tanh

    nc = bass.Bass("TRN2", target_bir_lowering=False, debug=False,
                   enable_asserts=False, num_devices=NCORES)

    d_xT = nc.dram_tensor("xT", (2, t_steps * BC), f16, kind="ExternalInput").ap()
    d_wihb = nc.dram_tensor("wihb", (2, H), f16, kind="ExternalInput").ap()
    d_WT = nc.dram_tensor("WT", (128, KC * H), f16, kind="ExternalInput").ap()
    d_lWT = nc.dram_tensor("lWT", (128, KC * NCLS), f16, kind="ExternalInput").ap()
    d_out = nc.dram_tensor("logitsT", (NCLS, BC), f32, kind="ExternalOutput").ap()

    with tile.TileContext(nc) as tc:
        with (
            tc.tile_pool(name="const", bufs=1) as cpool,
            tc.tile_pool(name="ps", bufs=1, space="PSUM") as ppool,
        ):
            xT_sb = cpool.tile([2, t_steps * BC], f16, tag="xT")
            wihb_sb = cpool.tile([2, H], f16, tag="wihb")
            WT_sb = cpool.tile([128, KC * H], f16, tag="WT")
            lWT_sb = cpool.tile([128, KC * NCLS], f16, tag="lWT")
            out_sb = cpool.tile([NCLS, BC], f32, tag="out")

            hh = [cpool.tile([128, 128], f16, tag=f"h{p}", name=f"h{p}")
                  for p in range(2)]
            stv = [cpool.tile([128, KC * BC], f16, tag=f"s{p}", name=f"s{p}")
                   for p in range(2)]
            pz = [ppool.tile([128, 128], f32, tag=f"pz{p}", name=f"pz{p}")
                  for p in range(2)]
            scr = cpool.tile([128, 512], f16, tag="scr")

            # dense PE prologue FIRST, reading a memset scratch tile
            # (values irrelevant, results discarded into pz[0] which the
            # x-term later resets): forces the core clock's fast state (see
            # module docstring) while the input DMAs stream in parallel.
            nc.vector.memset(scr[:, :], 0.25)
            for i in range(N_WARM):
                for j in range(4):
                    nc.tensor.matmul(
                        pz[0][j * BC:(j + 1) * BC, :],
                        scr[:, 0:BC],
                        scr[:, j * 128:(j + 1) * 128],
                        start=True, stop=True, skip_group_check=True,
                        tile_position=(0, j * BC),
                    )

            nc.sync.dma_start(out=xT_sb[:, :], in_=d_xT)
            nc.sync.dma_start(out=wihb_sb[:, :], in_=d_wihb)
            for kc in range(KC):
                nc.sync.dma_start(
                    out=WT_sb[:, kc * H:(kc + 1) * H],
                    in_=d_WT[:, kc * H:(kc + 1) * H],
                )
            nc.sync.dma_start(out=lWT_sb[:, :], in_=d_lWT)

            # gate matmuls: absorb each DMA queue's semaphore into the PE
            # clock once, so hot-loop matmuls need at most one wait.
            gates = [
                (xT_sb[0:2, 0:BC], xT_sb[0:2, 0:128]),
                (wihb_sb[0:2, 0:BC], wihb_sb[0:2, 0:128]),
            ]
            for kc in range(KC):
                gates.append(
                    (WT_sb[:, kc * H:kc * H + BC], WT_sb[:, kc * H:kc * H + 128])
                )
            gates.append((lWT_sb[:, 0:32], lWT_sb[:, 0:KC * NCLS]))
            for glhs, grhs in gates:
                w = min(grhs.shape[-1], 128)
                nc.tensor.matmul(pz[0][0:BC, 0:w], glhs, grhs[:, 0:w],
                                 start=True, stop=True, skip_group_check=True)

            def xmm(t):
                """x-term (start=True) for step t, per strip j."""
                p = t % 2
                xlhs = xT_sb[0:2, t * BC:(t + 1) * BC]
                last = t == 0  # no recurrence at t=0
                for j in range(4):
                    nc.tensor.matmul(
                        pz[p][j * BC:(j + 1) * BC, :],
                        xlhs,
                        wihb_sb[0:2, j * 128:(j + 1) * 128],
                        start=True,
                        stop=last,
                        skip_group_check=True,
                        tile_position=(0, j * BC),
                    )

            xmm(0)
            for t in range(t_steps):
                p, q = t % 2, 1 - (t % 2)
                if t > 0:
                    for kc in range(KC):
                        for j in range(4):
                            nc.tensor.matmul(
                                pz[p][j * BC:(j + 1) * BC, :],
                                stv[q][:, kc * BC:(kc + 1) * BC],
                                WT_sb[:, (kc * 4 + j) * 128:(kc * 4 + j + 1) * 128],
                                start=False,
                                stop=(kc == KC - 1),
                                skip_group_check=True,
                                tile_position=(0, j * BC),
                            )

                # next step's x-term fills the PE bubble during ACT/ST
                if t + 1 < t_steps:
                    xmm(t + 1)

                nc.scalar.activation(hh[p][:, :], pz[p][:, :], Tanh)
                nc.vector.transpose(stv[p][:, 0:64], hh[p][:, 0:64])
                nc.vector.transpose(stv[p][:, 64:128], hh[p][:, 64:128])

            # final linear head: logitsT[c, b] = sum_H lin_W[c, H] h[b, H]
            pl = (t_steps - 1) % 2
            pL = pz[1 - pl]
            for kc in range(KC):
                nc.tensor.matmul(
                    pL[0:NCLS, 0:BC],
                    lWT_sb[:, kc * NCLS:(kc + 1) * NCLS],
                    stv[pl][:, kc * BC:(kc + 1) * BC],
                    start=(kc == 0),
                    stop=(kc == KC - 1),
                    skip_group_check=True,
                )
            nc.vector.tensor_copy(out_sb[:, :], pL[0:NCLS, 0:BC])
            nc.sync.dma_start(out=d_out, in_=out_sb[:, :])

    _fold_updates_across_ldw(nc, mybir)
    _split_multi_waits(nc, mybir)
    return nc


def _fold_updates_across_ldw(nc, mybir):
    """Merge each run of same-flavor matmuls' sem updates onto the run's
    last member, skipping over interleaved wait-free/update-free Ldweights
    (the emission interleaves LDW+MM pairs, which defeats an adjacent-only
    fold). Every un-merged update is a ~26ns EVT write that delays the
    dependent ACT's wait resolution. Updates only move later within the
    engine's in-order stream, so all waiters still fire correctly."""
    def flavor(ins):
        return (getattr(ins, "start_tensor_calc", None),
                getattr(ins, "stop_tensor_calc", None),
                bool(getattr(ins, "is_transpose", False)))

    def waits(ins):
        si = getattr(ins, "sync_info", None)
        return list(getattr(si, "on_wait", []) or []) if si else []

    def updates(ins):
        si = getattr(ins, "sync_info", None)
        return list(getattr(si, "on_update", []) or []) if si else []

    def is_clean_ldw(ins):
        return (ins.__class__.__name__ == "InstLdweights"
                and not waits(ins) and not updates(ins))

    for b in nc.m.functions[0].blocks:
        ins_list = list(b.instructions)
        i = 0
        while i < len(ins_list):
            ins = ins_list[i]
            if ins.__class__.__name__ != "InstMatmult":
                i += 1
                continue
            run = [ins]
            j = i + 1
            last_j = i
            while j < len(ins_list):
                nxt = ins_list[j]
                if is_clean_ldw(nxt):
                    j += 1
                    continue
                if (nxt.__class__.__name__ == "InstMatmult"
                        and not waits(nxt) and flavor(nxt) == flavor(ins)):
                    run.append(nxt)
                    last_j = j
                    j += 1
                    continue
                break
            if len(run) > 1:
                merged = {}
                order = []
                for m in run:
                    for u in updates(m):
                        key = (str(u.sync_type), u.id, u.update_mode)
                        if key not in merged:
                            merged[key] = [u, 0]
                            order.append(key)
                        merged[key][1] += (u.update_value
                                           if u.update_value is not None else 1)
                last = run[-1]
                new_ups = [
                    mybir.SyncUpdate(
                        sync_type=merged[k][0].sync_type,
                        id=merged[k][0].id,
                        update_mode=("sem-add-imm"
                                     if merged[k][1] > 1
                                     and merged[k][0].update_mode == "sem-inc"
                                     else merged[k][0].update_mode),
                        ant_name=merged[k][0].ant_name,
                        update_value=merged[k][1],
                        update_reg=merged[k][0].update_reg,
                    )
                    for k in order
                ]
                for m in run[:-1]:
                    m.sync_info = mybir.SyncInfo(
                        on_wait=waits(m), on_update=[])
                last.sync_info = mybir.SyncInfo(
                    on_wait=waits(last), on_update=new_ups)
            i = last_j + 1


def _split_multi_waits(nc, mybir):
    """Walrus can pack only one sync wait into a HW instruction. Move any
    extra waits onto same-engine NoOps inserted right before (the engine's
    sequencer executes them in order, so semantics are unchanged)."""
    nid = 0
    for b in nc.m.functions[0].blocks:
        out = []
        changed = False
        for ins in b.instructions:
            si = getattr(ins, "sync_info", None)
            ws = list(getattr(si, "on_wait", []) or []) if si else []
            if len(ws) > 1:
                for w in ws[:-1]:
                    nid += 1
                    out.append(mybir.InstNoOp(
                        name=f"I-wsplit-{nid}",
                        engine=ins.engine,
                        sync_info=mybir.SyncInfo(on_wait=[w], on_update=[]),
                    ))
                ins.sync_info = mybir.SyncInfo(
                    on_wait=[ws[-1]], on_update=list(si.on_update or [])
                )
                changed = True
            out.append(ins)
        if changed:
            b.instructions = out


def _pack_inputs(inputs, order, W_ih, b_ih, W_hh, b_hh, lin_W, t_steps=T):
    """Host-side packing with the StreamTranspose layout permutation.

    H-index assignment: hh[32j+b, n] = h[b, 128j+n]. Slot kc contracts
    the H-subset {128r + 32kc + a : r, a}, whose stationary rows
    (k = 32r+a) come from STout[:, 32kc:32kc+32]. So rhs block (kc, j)
    [32r+a, n'] = W_hh[128j+n', 128r+32kc+a]."""
    x = np.asarray(inputs, np.float32)[:, np.asarray(order, np.int64)]
    x = np.ascontiguousarray(x[:, :t_steps])
    wihb = np.stack(
        [np.asarray(W_ih, np.float32)[:, 0],
         np.asarray(b_ih, np.float32) + np.asarray(b_hh, np.float32)]
    ).astype(np.float16)  # [2, H]

    Wf = np.asarray(W_hh, np.float32)          # [H_out, H_in]
    r = np.arange(4)[:, None]
    a = np.arange(32)[None, :]
    WT = np.empty((128, KC * H), np.float32)
    for kc in range(KC):
        hin = (128 * r + 32 * kc + a).reshape(128)   # [128] H_in indices
        for j in range(4):
            blk = Wf[128 * j:128 * (j + 1), hin].T   # [128(k), 128(n')]
            WT[:, (kc * 4 + j) * 128:(kc * 4 + j + 1) * 128] = blk
    WT = WT.astype(np.float16)

    lf = np.asarray(lin_W, np.float32)           # [NCLS, H]
    lWT = np.empty((128, KC * NCLS), np.float32)
    for kc in range(KC):
        hin = (128 * r + 32 * kc + a).reshape(128)
        lWT[:, kc * NCLS:(kc + 1) * NCLS] = lf[:, hin].T
    lWT = lWT.astype(np.float16)

    in_maps = []
    for c in range(NCORES):
        xc = x[c * BC:(c + 1) * BC]  # [BC, t]
        xT = np.ones((2, t_steps * BC), np.float16)
        xT[0] = xc.T.reshape(-1).astype(np.float16)
        in_maps.append({"xT": xT, "wihb": wihb, "WT": WT, "lWT": lWT})
    return in_maps


def _run(inputs, y, order, W_ih, b_ih, W_hh, b_hh, lin_W, lin_b, trace=False):
    from concourse import bass_utils

    key = T
    if key not in _BUILD_CACHE:
        _BUILD_CACHE[key] = _build(T)
    nc = _BUILD_CACHE[key]

    in_maps = _pack_inputs(inputs, order, W_ih, b_ih, W_hh, b_hh, lin_W, T)
    res = bass_utils.run_bass_kernel_spmd(
        nc, in_maps, core_ids=list(range(NCORES)), trace=trace
    )

    logits = np.empty((B, NCLS), np.float32)
    for c in range(NCORES):
        logits[c * BC:(c + 1) * BC] = res.results[c]["logitsT"].T
    logits = logits + np.asarray(lin_b, np.float32)[None, :]

    yv = np.asarray(y).astype(np.int64)
    m = logits.max(axis=1, keepdims=True)
    logp = logits - (np.log(np.exp(logits - m).sum(axis=1, keepdims=True)) + m)
    loss = np.float32(-logp[np.arange(B), yv].mean())
    correct = np.int32((logits.argmax(axis=1) == yv).sum())
    return (loss, correct), res


def kernel(inputs, y, order, W_ih, b_ih, W_hh, b_hh, lin_W, lin_b):
    out, _ = _run(inputs, y, order, W_ih, b_ih, W_hh, b_hh, lin_W, lin_b)
    return out


# revision 9
# speedup vs baseline: 1.2259x; 1.2259x over previous
"""Trainium2 Bass kernel for the pixel-RNN (tanh RNN, T=784, H=512, B=256).

Strategy: data-parallel over batch (32 samples per core, 8 cores); the
sequential 784-step loop runs locally per core; tiny classifier head on
device, log-softmax/loss/argmax on host from exact fp32 logits.

Per-step structure (all matmul operands fp16, PSUM accumulates f32):
  - z PSUM [128, 128]: 4-way column-strip packing; strip j (partitions
    32j..32j+32, tile_position (0,32j)) holds z[b, 128j+n].
  - x-term: per strip, one K=2 packed matmul [x_t; 1]^T @ [w_ih; b]
    (start=True), emitted one step early to fill the ACT/ST bubble.
  - recurrence: 4 slots (kc) x 4 packed strips, lhsT = a [128, 32]
    column slice of the block-transposed state (see below), rhs = a
    row-permuted W_hh^T block; each slot streams 128 cols (~100ns).
  - tanh: ONE ScalarE ACTIVATE [128,128] (~368ns). The ACT engine has
    exec-queue depth 0, so splitting the tanh serializes with an extra
    ~330ns const per piece - measured net loss.
  - layout fix: ONE DVE StreamTranspose pair (cols [0:64], [64:128]).
    ST semantics: STout[32r+a, 32c+d] = hh[32r+d, 32c+a], so the slice
    STout[:, 32kc:32kc+32] equals h[b, 128r + 32kc + a] - directly a
    valid matmul stationary for the H-subset {128r+32kc+a}. W_hh /
    lin_W rows are permuted host-side to match (exact relabeling).
    This replaces v1's PE transpose + 2 PSUM->SBUF vector copies
    (~800ns of chain latency) with ~390ns of DVE work; the 2-way ST
    split lets slots kc0/kc1 start ~70ns before ST2 retires (DVE
    queue depth 8 pipelines, unlike ACT).

Measured steady state (fast clock): ACT 368 + write-ack 215 + ST1 244 +
hop 54 + LDW 101 + 4 slot streams ~500 + tail 88 = ~1582ns/step. The
step is a latency chain, not a throughput problem: every engine idles
>60%; batch-splitting/core-pipelining cannot help because every piece's
cost is batch-size-independent and the 784 steps are strictly serial.

Clock-state pitfall (measured, not controllable): the whole core
(PE/ACT/DVE alike) runs exactly 1.2x slower on some runs (ACT 440 vs
367, MM 120 vs 94, period 1897 vs 1582). The state is fixed for an
entire device session/process from the first instruction on -- a ~36us
dense matmul prologue did NOT flip a slow run, and within one process
repeated executions all share the state. P(slow) ~ 0.25 per session
(likely which physical chip / HAM state the pool hands out). v1's
1457694 "good" runs vs 1747679 runs were the same lottery.

Sem plumbing: updates of same-flavor matmul runs are folded onto the
run's last member ACROSS interleaved wait-free LDWEIGHTS (the LDW+MM
emission interleave otherwise defeats folding and each MM's ~26ns EVT
write delays the ACT's wait resolution); multi-wait instructions are
split so each HW instruction carries exactly one wait (codegen limit).

History: v1 (PE transpose + copies) 1457694ns fast-state / 1747679
slow-state -> v2 (StreamTranspose layout + fold-across-LDW) 1259299ns
fast-state / ~1511000 slow-state.
"""

import sys

if "/opt/trn_rl_repo" not in sys.path:
    sys.path.insert(0, "/opt/trn_rl_repo")

import numpy as np

B, T, H, NCLS = 256, 784, 512, 10
NCORES = 8
BC = B // NCORES   # 32 samples per core
KC = H // 128      # 4 contraction slots

_BUILD_CACHE = {}


def _build(t_steps=T):
    import concourse.bass as bass
    import concourse.mybir as mybir
    from concourse import tile

    f32 = mybir.dt.float32
    f16 = mybir.dt.float16
    Tanh = mybir.ActivationFunctionType.Tanh

    nc = bass.Bass("TRN2", target_bir_lowering=False, debug=False,
                   enable_asserts=False, num_devices=NCORES)

    d_xT = nc.dram_tensor("xT", (2, t_steps * BC), f16, kind="ExternalInput").ap()
    d_wihb = nc.dram_tensor("wihb", (2, H), f16, kind="ExternalInput").ap()
    d_WT = nc.dram_tensor("WT", (128, KC * H), f16, kind="ExternalInput").ap()
    d_lWT = nc.dram_tensor("lWT", (128, KC * NCLS), f16, kind="ExternalInput").ap()
    d_out = nc.dram_tensor("logitsT", (NCLS, BC), f32, kind="ExternalOutput").ap()

    with tile.TileContext(nc) as tc:
        with (
            tc.tile_pool(name="const", bufs=1) as cpool,
            tc.tile_pool(name="ps", bufs=1, space="PSUM") as ppool,
        ):
            xT_sb = cpool.tile([2, t_steps * BC], f16, tag="xT")
            wihb_sb = cpool.tile([2, H], f16, tag="wihb")
            WT_sb = cpool.tile([128, KC * H], f16, tag="WT")
            lWT_sb = cpool.tile([128, KC * NCLS], f16, tag="lWT")
            out_sb = cpool.tile([NCLS, BC], f32, tag="out")

            hh = [cpool.tile([128, 128], f16, tag=f"h{p}", name=f"h{p}")
                  for p in range(2)]
            stv = [cpool.tile([128, KC * BC], f16, tag=f"s{p}", name=f"s{p}")
                   for p in range(2)]
            pz = [ppool.tile([128, 128], f32, tag=f"pz{p}", name=f"pz{p}")
                  for p in range(2)]

            nc.sync.dma_start(out=xT_sb[:, :], in_=d_xT)
            nc.sync.dma_start(out=wihb_sb[:, :], in_=d_wihb)
            for kc in range(KC):
                nc.sync.dma_start(
                    out=WT_sb[:, kc * H:(kc + 1) * H],
                    in_=d_WT[:, kc * H:(kc + 1) * H],
                )
            nc.sync.dma_start(out=lWT_sb[:, :], in_=d_lWT)

            # gate matmuls: absorb each DMA queue's semaphore into the PE
            # clock once, so hot-loop matmuls need at most one wait.
            gates = [
                (xT_sb[0:2, 0:BC], xT_sb[0:2, 0:128]),
                (wihb_sb[0:2, 0:BC], wihb_sb[0:2, 0:128]),
            ]
            for kc in range(KC):
                gates.append(
                    (WT_sb[:, kc * H:kc * H + BC], WT_sb[:, kc * H:kc * H + 128])
                )
            gates.append((lWT_sb[:, 0:32], lWT_sb[:, 0:KC * NCLS]))
            for glhs, grhs in gates:
                w = min(grhs.shape[-1], 128)
                nc.tensor.matmul(pz[0][0:BC, 0:w], glhs, grhs[:, 0:w],
                                 start=True, stop=True, skip_group_check=True)

            def xmm(t):
                """x-term (start=True) for step t, per strip j."""
                p = t % 2
                xlhs = xT_sb[0:2, t * BC:(t + 1) * BC]
                last = t == 0  # no recurrence at t=0
                for j in range(4):
                    nc.tensor.matmul(
                        pz[p][j * BC:(j + 1) * BC, :],
                        xlhs,
                        wihb_sb[0:2, j * 128:(j + 1) * 128],
                        start=True,
                        stop=last,
                        skip_group_check=True,
                        tile_position=(0, j * BC),
                    )

            xmm(0)
            for t in range(t_steps):
                p, q = t % 2, 1 - (t % 2)
                if t > 0:
                    for kc in range(KC):
                        for j in range(4):
                            nc.tensor.matmul(
                                pz[p][j * BC:(j + 1) * BC, :],
                                stv[q][:, kc * BC:(kc + 1) * BC],
                                WT_sb[:, (kc * 4 + j) * 128:(kc * 4 + j + 1) * 128],
                                start=False,
                                stop=(kc == KC - 1),
                                skip_group_check=True,
                                tile_position=(0, j * BC),
                            )

                # next step's x-term fills the PE bubble during ACT/ST
                if t + 1 < t_steps:
                    xmm(t + 1)

                nc.scalar.activation(hh[p][:, :], pz[p][:, :], Tanh)
                nc.vector.transpose(stv[p][:, 0:64], hh[p][:, 0:64])
                nc.vector.transpose(stv[p][:, 64:128], hh[p][:, 64:128])

            # final linear head: logitsT[c, b] = sum_H lin_W[c, H] h[b, H]
            pl = (t_steps - 1) % 2
            pL = pz[1 - pl]
            for kc in range(KC):
                nc.tensor.matmul(
                    pL[0:NCLS, 0:BC],
                    lWT_sb[:, kc * NCLS:(kc + 1) * NCLS],
                    stv[pl][:, kc * BC:(kc + 1) * BC],
                    start=(kc == 0),
                    stop=(kc == KC - 1),
                    skip_group_check=True,
                )
            nc.vector.tensor_copy(out_sb[:, :], pL[0:NCLS, 0:BC])
            nc.sync.dma_start(out=d_out, in_=out_sb[:, :])

    _fold_updates_across_ldw(nc, mybir)
    _split_multi_waits(nc, mybir)
    return nc


def _fold_updates_across_ldw(nc, mybir):
    """Merge each run of same-flavor matmuls' sem updates onto the run's
    last member, skipping over interleaved wait-free/update-free Ldweights
    (the emission interleaves LDW+MM pairs, which defeats an adjacent-only
    fold). Every un-merged update is a ~26ns EVT write that delays the
    dependent ACT's wait resolution. Updates only move later within the
    engine's in-order stream, so all waiters still fire correctly."""
    def flavor(ins):
        return (getattr(ins, "start_tensor_calc", None),
                getattr(ins, "stop_tensor_calc", None),
                bool(getattr(ins, "is_transpose", False)))

    def waits(ins):
        si = getattr(ins, "sync_info", None)
        return list(getattr(si, "on_wait", []) or []) if si else []

    def updates(ins):
        si = getattr(ins, "sync_info", None)
        return list(getattr(si, "on_update", []) or []) if si else []

    def is_clean_ldw(ins):
        return (ins.__class__.__name__ == "InstLdweights"
                and not waits(ins) and not updates(ins))

    for b in nc.m.functions[0].blocks:
        ins_list = list(b.instructions)
        i = 0
        while i < len(ins_list):
            ins = ins_list[i]
            if ins.__class__.__name__ != "InstMatmult":
                i += 1
                continue
            run = [ins]
            j = i + 1
            last_j = i
            while j < len(ins_list):
                nxt = ins_list[j]
                if is_clean_ldw(nxt):
                    j += 1
                    continue
                if (nxt.__class__.__name__ == "InstMatmult"
                        and not waits(nxt) and flavor(nxt) == flavor(ins)):
                    run.append(nxt)
                    last_j = j
                    j += 1
                    continue
                break
            if len(run) > 1:
                merged = {}
                order = []
                for m in run:
                    for u in updates(m):
                        key = (str(u.sync_type), u.id, u.update_mode)
                        if key not in merged:
                            merged[key] = [u, 0]
                            order.append(key)
                        merged[key][1] += (u.update_value
                                           if u.update_value is not None else 1)
                last = run[-1]
                new_ups = [
                    mybir.SyncUpdate(
                        sync_type=merged[k][0].sync_type,
                        id=merged[k][0].id,
                        update_mode=("sem-add-imm"
                                     if merged[k][1] > 1
                                     and merged[k][0].update_mode == "sem-inc"
                                     else merged[k][0].update_mode),
                        ant_name=merged[k][0].ant_name,
                        update_value=merged[k][1],
                        update_reg=merged[k][0].update_reg,
                    )
                    for k in order
                ]
                for m in run[:-1]:
                    m.sync_info = mybir.SyncInfo(
                        on_wait=waits(m), on_update=[])
                last.sync_info = mybir.SyncInfo(
                    on_wait=waits(last), on_update=new_ups)
            i = last_j + 1


def _split_multi_waits(nc, mybir):
    """Walrus can pack only one sync wait into a HW instruction. Move any
    extra waits onto same-engine NoOps inserted right before (the engine's
    sequencer executes them in order, so semantics are unchanged)."""
    nid = 0
    for b in nc.m.functions[0].blocks:
        out = []
        changed = False
        for ins in b.instructions:
            si = getattr(ins, "sync_info", None)
            ws = list(getattr(si, "on_wait", []) or []) if si else []
            if len(ws) > 1:
                for w in ws[:-1]:
                    nid += 1
                    out.append(mybir.InstNoOp(
                        name=f"I-wsplit-{nid}",
                        engine=ins.engine,
                        sync_info=mybir.SyncInfo(on_wait=[w], on_update=[]),
                    ))
                ins.sync_info = mybir.SyncInfo(
                    on_wait=[ws[-1]], on_update=list(si.on_update or [])
                )
                changed = True
            out.append(ins)
        if changed:
            b.instructions = out


def _pack_inputs(inputs, order, W_ih, b_ih, W_hh, b_hh, lin_W, t_steps=T):
    """Host-side packing with the StreamTranspose layout permutation.

    H-index assignment: hh[32j+b, n] = h[b, 128j+n]. Slot kc contracts
    the H-subset {128r + 32kc + a : r, a}, whose stationary rows
    (k = 32r+a) come from STout[:, 32kc:32kc+32]. So rhs block (kc, j)
    [32r+a, n'] = W_hh[128j+n', 128r+32kc+a]."""
    x = np.asarray(inputs, np.float32)[:, np.asarray(order, np.int64)]
    x = np.ascontiguousarray(x[:, :t_steps])
    wihb = np.stack(
        [np.asarray(W_ih, np.float32)[:, 0],
         np.asarray(b_ih, np.float32) + np.asarray(b_hh, np.float32)]
    ).astype(np.float16)  # [2, H]

    Wf = np.asarray(W_hh, np.float32)          # [H_out, H_in]
    r = np.arange(4)[:, None]
    a = np.arange(32)[None, :]
    WT = np.empty((128, KC * H), np.float32)
    for kc in range(KC):
        hin = (128 * r + 32 * kc + a).reshape(128)   # [128] H_in indices
        for j in range(4):
            blk = Wf[128 * j:128 * (j + 1), hin].T   # [128(k), 128(n')]
            WT[:, (kc * 4 + j) * 128:(kc * 4 + j + 1) * 128] = blk
    WT = WT.astype(np.float16)

    lf = np.asarray(lin_W, np.float32)           # [NCLS, H]
    lWT = np.empty((128, KC * NCLS), np.float32)
    for kc in range(KC):
        hin = (128 * r + 32 * kc + a).reshape(128)
        lWT[:, kc * NCLS:(kc + 1) * NCLS] = lf[:, hin].T
    lWT = lWT.astype(np.float16)

    in_maps = []
    for c in range(NCORES):
        xc = x[c * BC:(c + 1) * BC]  # [BC, t]
        xT = np.ones((2, t_steps * BC), np.float16)
        xT[0] = xc.T.reshape(-1).astype(np.float16)
        in_maps.append({"xT": xT, "wihb": wihb, "WT": WT, "lWT": lWT})
    return in_maps


def _run(inputs, y, order, W_ih, b_ih, W_hh, b_hh, lin_W, lin_b, trace=False):
    from concourse import bass_utils

    key = T
    if key not in _BUILD_CACHE:
        _BUILD_CACHE[key] = _build(T)
    nc = _BUILD_CACHE[key]

    in_maps = _pack_inputs(inputs, order, W_ih, b_ih, W_hh, b_hh, lin_W, T)
    res = bass_utils.run_bass_kernel_spmd(
        nc, in_maps, core_ids=list(range(NCORES)), trace=trace
    )

    logits = np.empty((B, NCLS), np.float32)
    for c in range(NCORES):
        logits[c * BC:(c + 1) * BC] = res.results[c]["logitsT"].T
    logits = logits + np.asarray(lin_b, np.float32)[None, :]

    yv = np.asarray(y).astype(np.int64)
    m = logits.max(axis=1, keepdims=True)
    logp = logits - (np.log(np.exp(logits - m).sum(axis=1, keepdims=True)) + m)
    loss = np.float32(-logp[np.arange(B), yv].mean())
    correct = np.int32((logits.argmax(axis=1) == yv).sum())
    return (loss, correct), res


def kernel(inputs, y, order, W_ih, b_ih, W_hh, b_hh, lin_W, lin_b):
    out, _ = _run(inputs, y, order, W_ih, b_ih, W_hh, b_hh, lin_W, lin_b)
    return out
